# revision 8
# baseline (speedup 1.0000x reference)
"""Single-head causal attention (B=8, T=2048, C=1024, head_dim=64) on 8 TRN2 NeuronCores.

Sharding: data-parallel over batch -- one batch element per core, qkv weights
replicated. Host prep per core: x[b] is transposed to [C, T] and cast to fp16
(PE streams fp16 at 1 cycle/row vs 4 for fp32; fp16's 11-bit mantissa keeps the
end-to-end error ~1e-3, and all PSUM accumulation stays fp32).

Device layout (everything kept transposed so no P-tile transposes are needed):
  kqT  = Wkq^T x^T + b_kq     [128, T]   (k rows 0:64, q rows 64:128)
  vT   = Wv^T x^T             [64, T] -> v1 [s,65] via DMA-transpose (+ones col)
  ST_j = K_j Q^T              [128 s, t] per 128-row s-chunk, causal t >= s only
  P^T  = exp(0.125 * ST)      masked upper-tri on the diagonal block
  outT[g] += v1_j^T P^T_j     [65, 512] per 512-col t-group (row 64 = denom)
  out  = PE-transpose(outT) -> [t, 65]; out[:, :64] / denom + b_v
"""

import numpy as np

import concourse.bass as bass
import concourse.mybir as mybir
from concourse import bacc
from concourse.bass import ts
from concourse.bass_utils import run_bass_kernel_spmd
from concourse.masks import make_identity, make_upper_triangular
from concourse.tile import TileContext

B, T, C = 8, 2048, 1024
HD = 64
N_CORES = 8
NJ = C // 128  # contraction chunks for the qkv projection
NT = T // 128  # 128-row tiles along T
NG = T // 512  # 512-col groups along T
FP16 = mybir.dt.float16
F32 = mybir.dt.float32
EXP = mybir.ActivationFunctionType.Exp


def build_nc() -> bass.Bass:
    nc = bacc.Bacc(None, target_bir_lowering=False)
    xt = nc.declare_dram_parameter("xt", [C, T], FP16, isOutput=False)
    w = nc.declare_dram_parameter("w", [C, 3 * HD], FP16, isOutput=False)
    wb = nc.declare_dram_parameter("wb", [1, 3 * HD], FP16, isOutput=False)
    out = nc.declare_dram_parameter("out", [T, HD], F32, isOutput=True)

    with TileContext(nc) as tc:
        with (
            tc.tile_pool(name="consts", bufs=1) as consts,
            tc.tile_pool(name="xtp", bufs=NJ) as xtp,
            tc.tile_pool(name="kqv", bufs=1) as kqv,
            tc.tile_pool(name="ptp", bufs=3) as ptp,
            tc.tile_pool(name="epi", bufs=2) as epi,
            tc.tile_pool(name="ps", bufs=8, space=bass.MemorySpace.PSUM) as ps,
        ):
            # --- constants ---
            w_sb = consts.tile([128, NJ, 3 * HD], FP16)
            nc.sync.dma_start(out=w_sb[:], in_=w[:, :].rearrange("(n p) m -> p n m", p=128))
            wb_sb = consts.tile([1, 3 * HD], FP16)
            nc.sync.dma_start(out=wb_sb[:], in_=wb[:, :])
            ones_sb = consts.tile([1, T], FP16)
            nc.vector.memset(ones_sb[:], 1.0)
            mask_sb = consts.tile([128, 128], FP16)
            make_upper_triangular(nc, mask_sb[:], val=1.0, diag=True)
            ident = consts.tile([128, 128], F32)
            make_identity(nc, ident[:])
            ident_h = consts.tile([128, 128], FP16)
            make_identity(nc, ident_h[:])
            # sel[c, m] = 1 iff c == m + 64: extracts partitions 64:128 -> 0:64
            sel = consts.tile([128, 64], FP16)
            nc.gpsimd.memset(sel[:], 0.0)
            nc.gpsimd.affine_select(
                out=sel[:], in_=sel[:],
                compare_op=mybir.AluOpType.not_equal,
                fill=1.0, base=-64, pattern=[[-1, 64]], channel_multiplier=1,
            )

            # --- load x^T in 128-partition chunks ---
            xts = []
            for j in range(NJ):
                xt_t = xtp.tile([128, T], FP16, tag="xt")
                nc.sync.dma_start(out=xt_t[:], in_=xt[ts(j, 128), :])
                xts.append(xt_t)

            # --- qkv projection, contraction accumulated in PSUM ---
            kq_acc = [ps.tile([128, 512], F32, tag="ps", name=f"kq_acc{n}") for n in range(NG)]
            v_acc = [ps.tile([64, 512], F32, tag="ps", name=f"v_acc{n}") for n in range(NG)]
            for j in range(NJ):
                first = j == 0
                for n in range(NG):
                    nc.tensor.matmul(
                        kq_acc[n][:], w_sb[:, j, 0:128], xts[j][:, ts(n, 512)],
                        start=first, stop=False,
                    )
                for n in range(NG):
                    nc.tensor.matmul(
                        v_acc[n][:], w_sb[:, j, 128:192], xts[j][:, ts(n, 512)],
                        start=first, stop=False,
                    )
            # bias via an augmented K=1 chunk: ones row (rhs) x bias row (lhsT)
            for n in range(NG):
                nc.tensor.matmul(
                    kq_acc[n][:], wb_sb[:, 0:128], ones_sb[:, ts(n, 512)],
                    start=False, stop=True,
                )
                nc.tensor.matmul(
                    v_acc[n][:], wb_sb[:, 128:192], ones_sb[:, ts(n, 512)],
                    start=False, stop=True,
                )

            kqT = kqv.tile([128, T], FP16)
            vT = kqv.tile([128, T], FP16)
            nc.vector.memset(vT[64:128, :], 0.0)
            for n in range(NG):
                nc.vector.tensor_copy(kqT[:, ts(n, 512)], kq_acc[n][:])
                nc.vector.tensor_copy(vT[0:64, ts(n, 512)], v_acc[n][:])

            # q must sit at base partition 0 to feed matmuls: PE row-extract
            qT = kqv.tile([64, T], FP16)
            for n in range(NG):
                qp = ps.tile([64, 512], F32, tag="ps", name=f"qp{n}")
                nc.tensor.matmul(qp[:], sel[:], kqT[:, ts(n, 512)], start=True, stop=True)
                nc.vector.tensor_copy(qT[:, ts(n, 512)], qp[:])

            # v in [s, hd] layout plus a ones column (softmax-denominator trick),
            # via PE transposes of the zero-padded vT
            v1 = kqv.tile([128, NT, 80], FP16)
            for i in range(NT):
                tpv = ps.tile([128, 128], FP16, tag="ps", name=f"tpv{i}")
                nc.tensor.transpose(tpv[:], vT[:, ts(i, 128)], ident_h[:])
                nc.vector.tensor_copy(v1[:, i, 0:HD], tpv[:, 0:HD])
                nc.vector.memset(v1[:, i, HD:HD + 1], 1.0)

            # --- attention ---
            outT_acc = [ps.tile([65, 512], F32, tag="ps", name=f"outT_acc{g}") for g in range(NG)]
            for j in range(NT):
                ptj = ptp.tile([128, T], FP16, tag="pt")
                for g in range(j // 4, NG):
                    a, b2 = max(128 * j, 512 * g), 512 * (g + 1)
                    stp = ps.tile([128, b2 - a], F32, tag="ps")
                    nc.tensor.matmul(
                        stp[:], kqT[0:64, ts(j, 128)], qT[:, a:b2],
                        start=True, stop=True,
                    )
                    nc.scalar.activation(ptj[:, a:b2], stp[:], EXP, scale=0.125)
                # zero the below-diagonal (s > t) entries of the diagonal block
                nc.vector.tensor_mul(ptj[:, ts(j, 128)], ptj[:, ts(j, 128)], mask_sb[:])
                for g in range(j // 4, NG):
                    a, b2 = max(128 * j, 512 * g), 512 * (g + 1)
                    nc.tensor.matmul(
                        outT_acc[g][:, a - 512 * g:512], v1[:, j, 0:65], ptj[:, a:b2],
                        start=(j == 0), stop=(j == 4 * g + 3),
                    )

            # --- epilogue: transpose outT, divide by the softmax denom, add b_v ---
            for g in range(NG):
                eo = epi.tile([128, 512], F32, tag="eo")
                nc.vector.memset(eo[64:128, :], 0.0)
                nc.vector.tensor_copy(eo[0:65, :], outT_acc[g][:])
                for l in range(4):
                    i = 4 * g + l
                    tp = ps.tile([128, 128], F32, tag="ps")
                    nc.tensor.transpose(tp[:], eo[:, ts(l, 128)], ident[:])
                    rcp = epi.tile([128, 1], F32, tag="rcp")
                    nc.vector.reciprocal(rcp[:], tp[:, HD:HD + 1])
                    ob = epi.tile([128, HD], F32, tag="ob")
                    nc.vector.tensor_scalar_mul(ob[:], tp[:, 0:HD], rcp[:])
                    nc.sync.dma_start(out=out[ts(i, 128), :], in_=ob[:])
    nc.compile()
    return nc


_NC_CACHE = None


def _get_nc() -> bass.Bass:
    global _NC_CACHE
    if _NC_CACHE is None:
        _NC_CACHE = build_nc()
    return _NC_CACHE


def make_in_maps(x: np.ndarray, W: np.ndarray, b: np.ndarray) -> list[dict]:
    w16 = np.ascontiguousarray(W.astype(np.float16))
    wb16 = np.ascontiguousarray(b.astype(np.float16).reshape(1, 3 * HD))
    in_maps = []
    for core in range(N_CORES):
        xt = np.ascontiguousarray(x[core].astype(np.float16).T)
        in_maps.append({"xt": xt, "w": w16, "wb": wb16})
    return in_maps


def run(x, W, b, trace: bool = False):
    """Returns (output [B, T, HD] fp32, BassKernelResults)."""
    x, W, b = np.asarray(x), np.asarray(W), np.asarray(b)
    nc = _get_nc()
    res = run_bass_kernel_spmd(nc, make_in_maps(x, W, b), list(range(N_CORES)), trace=trace)
    out = np.stack([res.results[i]["out"] for i in range(N_CORES)], axis=0)
    return out.astype(np.float32), res


def kernel(x, W, b) -> np.ndarray:
    out, _ = run(x, W, b)
    return out
